# revision 52
# baseline (speedup 1.0000x reference)
"""Trainium2 Bass kernel for nn_CrossCorrV2.

Math: with P = nd0*nd1 = 100 patches and OUT_CHANNEL = 100, top_k over the
patch axis returns *all* patches, so mean(top_k) == mean over patches.  Both
the grouped conv (linear in the filter) and the bilinear resize (linear map)
commute with that mean, so the whole module collapses per sample to

    out[b] = resize129->128( corr2d(x1n[b], K[b]) ) / (hh*ww*P)

where x1n/x2n are channel-L2-normalized and K[b][c, dy, dx] =
sum_{grid} x2n[b, c, i0*6+dy, i1*6+dx] (a 6x6 fold of the normalized x2).

Per-core pipeline (1 sample per NeuronCore, 8 cores).  This environment
executes NEFFs at a fixed ~20-40us per *instruction*, so the kernel is
instruction-count optimized: every fold/shift is expressed as a single
strided-AP instruction and all PE transposes are replaced by xbar DMA
transposes plus free-dim reductions.

  - x2 ([c, pix]): square -> xbar -> reduce -> sqrt -> recip -> DRAM
    broadcast -> one normalize multiply -> ONE nested-AP tensor_reduce
    (both grid axes at once) -> Kb [64, 48] bf16.
  - x1 norms: two squares (ACT/DVE) -> two xbar transposes -> two free
    reduces -> sqrt -> recip -> nrmT [j, r].
  - conv: Kb stationary, 32 matmuls streaming x1 [64, 512] chunks ->
    Z [48, 16384] PSUM -> 4 copy-casts to bf16 -> ONE xbar transpose ->
    ZTt [j, r, o] -> one broadcast multiply by nrmT writing into a
    zero-padded [j, o, rpad=134] layout.
  - col2im dy-fold: ONE tensor_reduce with a stride-805 virtual axis
    (805 = 6*134 + 1 couples the o-step and the r-shift).
  - I-resize (2-tap) in free dim: 3 vector ops -> bf16 [j, dx, Io].
  - ONE xbar transpose into zero-padded [Io, dx, jpad=134]; col2im
    dx-fold: ONE tensor_reduce with a stride-135 virtual axis.
  - J-resize in free dim: 3 vector ops -> out [I, J] row-major, 1 DMA.

A post-legalization pass dedups the per-matmul InstLdweights that tile
legalization inserts (the PE array retains stationary weights across
matmuls), collapsing 32 identical weight loads into 1.
"""

import os
import sys
import functools

import numpy as np

for _p in ("/opt/trn_rl_repo", "/root/.axon_site/_ro/trn_rl_repo"):
    if os.path.isdir(_p) and _p not in sys.path:
        sys.path.insert(0, _p)

import ml_dtypes
import concourse.bass as bass
import concourse.mybir as mybir
import concourse.tile as tile
import concourse.tile_sem_assignment as tsa
from concourse.vector_clock import ScopedClock

# Optimization switches (module-level so a bisect harness can flip them;
# the lru_cache on build_program must be cleared after changing any).
OPT = {
    "one_queue": True,     # collapse HW-DGE queue round-robin to 1 queue
    "dedup_ldw": True,     # drop repeated identical InstLdweights
    "reduce_waits": True,  # transitive reduction of semaphore waits
    "strip": True,         # preamble regs / dead consts / barrier tail
    "entry_barrier": False,
    "merge_blocks": True,
    "big_xbar": True,      # one 16384-wide Z transpose instead of 4
    "sem_clear": False,    # rely on the runtime resetting sems per execution
}

# Collapse the HW-DGE queue round-robin to one queue: every DMA then ticks
# the same DMAHW0 semaphore, so the TileContext tail drain needs 1 wait
# instead of 8 (each wait is a whole Drain instruction in this walrus).
_ORIG_NUM_HWDGE = tsa.NUM_HWDGE_SEMS


def _apply_queue_opt():
    tsa.NUM_HWDGE_SEMS = 1 if OPT["one_queue"] else _ORIG_NUM_HWDGE

BF16 = ml_dtypes.bfloat16
F32 = mybir.dt.float32
BF = mybir.dt.bfloat16

B, C, H, W = 8, 64, 128, 128
h2, w2 = 60, 60
PS = 6                      # patch size (hh == ww == 6)
KO = PS * PS                # 36 filter taps
KOP = 48                    # padded taps (xbar transpose needs mult of 16)
NPATCH = 100
SCALE = 1.0 / (PS * PS * NPATCH)   # 1/3600
NCORES = 8

HWPIX = H * W               # 16384
XHALF = HWPIX // 2          # 8192
PIX2 = h2 * w2              # 3600
PIX2P = 29 * 128            # 3712 (x2 pixels padded to xbar granule)
RP = H + PS                 # 134: padded r pitch (129 outputs + 5 slack)
IP = H + 1                  # 129 conv output rows/cols


class _PhaseStop(Exception):
    def __init__(self, ap):
        self.ap = ap


def _patch_tile_drain():
    """Replace the TileContext tail (drain + 2 all-engine barriers + sem
    clear) with the minimal equivalent: ONE Pool no-op carrying the final-
    clock waits for every semaphore (split later into 1-wait no-ops by
    _split_excess_waits), then the DGE reset + semaphore clear on Pool.
    Engine streams quiesce transitively (every instruction has a consumer
    path into the final waits), so no gather/release barrier is needed."""
    if getattr(tile.TileContext, "_drain_patched", False):
        return

    def _patched(self, tick_clock, wait_clock):
        nc = self.nc
        nop = mybir.InstNoOp(
            name="tail_wait", engine=mybir.EngineType.Pool, ins=[], outs=[]
        )
        nop_bi = nc.gpsimd.add_instruction(nop)
        wait_clock.add_sem_waits(
            nop_bi.ins, ScopedClock({None: tick_clock.global_clock})
        )
        popped = nc._tile_sem_poison_stack.pop()
        assert popped is self._sem_poison
        if OPT["sem_clear"]:
            nc.clear_and_free_semaphores(list(self.sems.allocated().values()))

    tile.TileContext._drain_and_barrier = _patched
    tile.TileContext._drain_patched = True


def _split_excess_waits(nc):
    """Walrus here rejects >1 sync wait per instruction; move excess waits
    onto same-engine NoOps spliced immediately before the instruction."""
    n = 0
    for f in nc.m.functions:
        for bb in f.blocks:
            out = []
            for ins in bb.instructions:
                si = ins.sync_info
                if si is not None and si.on_wait and len(si.on_wait) > 1:
                    waits = list(si.on_wait)
                    for j, w in enumerate(waits[:-1]):
                        nop = mybir.InstNoOp(
                            name=f"{ins.name}_sw{j}",
                            engine=ins.engine,
                            ins=[],
                            outs=[],
                            sync_info=mybir.SyncInfo(on_wait=[w], on_update=[]),
                        )
                        out.append(nop)
                        n += 1
                    ins.sync_info = mybir.SyncInfo(
                        on_wait=[waits[-1]], on_update=list(si.on_update or [])
                    )
                out.append(ins)
            bb.instructions = out
    return n


def _dedup_ldweights(nc):
    """Tile legalization pairs every InstMatmult with an InstLdweights even
    when the stationary weights are unchanged.  The PE array retains loaded
    weights across matmuls, so drop consecutive identical loads, folding the
    removed load's sync_info into the following instruction (the matmult).
    Run BEFORE _split_excess_waits so merged waits get re-split if needed."""
    n = 0
    for f in nc.m.functions:
        for bb in f.blocks:
            out = []
            last_sig = None
            for ins in bb.instructions:
                if isinstance(ins, mybir.InstLdweights):
                    sig = (
                        str(ins.ins),
                        getattr(ins, "is_transpose", None),
                        str(getattr(ins, "tile_position", None)),
                        str(getattr(ins, "tile_size", None)),
                        getattr(ins, "perf_mode", None),
                    )
                    if sig == last_sig:
                        # redundant load: push its sync onto the next inst
                        si = ins.sync_info
                        if si is not None and (si.on_wait or si.on_update):
                            ins._pending_sync = si  # stash
                            out.append(("PENDING", si))
                        n += 1
                        continue
                    last_sig = sig
                    out.append(ins)
                else:
                    if isinstance(ins, mybir.InstMatmult) and getattr(
                        ins, "is_transpose", False
                    ):
                        last_sig = None  # transposes clobber the array
                    out.append(ins)
            # fold pending syncs into the next real instruction
            merged = []
            pend_waits, pend_upds = [], []
            for item in out:
                if isinstance(item, tuple) and item[0] == "PENDING":
                    si = item[1]
                    pend_waits.extend(list(si.on_wait or []))
                    pend_upds.extend(list(si.on_update or []))
                    continue
                ins = item
                if pend_waits or pend_upds:
                    si = ins.sync_info
                    w = list(si.on_wait or []) if si else []
                    u = list(si.on_update or []) if si else []
                    ins.sync_info = mybir.SyncInfo(
                        on_wait=pend_waits + w, on_update=pend_upds + u
                    )
                    pend_waits, pend_upds = [], []
                merged.append(ins)
            assert not pend_waits and not pend_upds
            bb.instructions = merged
    return n


_INC_MODES = ("sem-inc", "sem-add-imm")


def _reduce_waits(nc):
    """Transitive reduction of semaphore waits.  A wait (S >= v) is dropped
    when the waiting engine's happens-before clock already covers it —
    through program order on the engine plus the clocks carried by
    previously-joined waits.  Sound because sems are monotone counters and
    the scheduled instruction order lists every (S, v) producer before any
    consumer referencing it."""
    engine_clock = {}
    sem_count = {}
    sem_clocks = {}
    n = 0
    dma_types = (mybir.InstDMACopy, mybir.InstDmaTransposeAnt)
    for f in nc.m.functions:
        for bb in f.blocks:
            for ins in bb.instructions:
                V = engine_clock.setdefault(ins.engine, {})
                is_dma = isinstance(ins, dma_types)
                my_dma_sems = set()
                si = ins.sync_info
                if si is not None and is_dma and si.on_update:
                    my_dma_sems = {
                        u.id for u in si.on_update
                        if getattr(u, "update_mode", "") in _INC_MODES
                    }
                keep_all = OPT["sem_clear"] and (
                    ins.name == "tail_wait"
                    or ins.name.startswith("tail_wait_sw"))
                if si is not None and si.on_wait:
                    # fixpoint: a wait is redundant if covered by the engine
                    # clock plus the producer clocks of the OTHER kept waits
                    ge, keep_other = [], []
                    for w in si.on_wait:
                        (ge if getattr(w, "wait_mode", "") == "sem-ge-imm"
                         else keep_other).append(w)
                    def _pclock(w):
                        pc = dict(sem_clocks.get(w.id, {}).get(w.wait_value, {}))
                        pc[w.id] = max(pc.get(w.id, 0), w.wait_value)
                        return pc
                    kept = list(ge)
                    if not keep_all:
                        changed = True
                        while changed:
                            changed = False
                            for w in list(kept):
                                cov = dict(V)
                                for w2 in kept:
                                    if w2 is w:
                                        continue
                                    for s2, v2 in _pclock(w2).items():
                                        if cov.get(s2, 0) < v2:
                                            cov[s2] = v2
                                implied = cov.get(w.id, 0) >= w.wait_value
                                if implied:
                                    kept.remove(w)
                                    n += 1
                                    changed = True
                                    break
                    for w in kept:
                        for s2, v2 in _pclock(w).items():
                            if V.get(s2, 0) < v2:
                                V[s2] = v2
                    if keep_all:
                        # still record coverage for downstream bookkeeping
                        pass
                    if len(kept) + len(keep_other) != len(si.on_wait):
                        ins.sync_info = mybir.SyncInfo(
                            on_wait=keep_other + kept,
                            on_update=list(si.on_update or []),
                        )
                si = ins.sync_info
                if si is not None and si.on_update:
                    for u in si.on_update:
                        sid = u.id
                        if getattr(u, "update_mode", "") not in _INC_MODES:
                            sem_count[sid] = None  # opaque: stop tracking
                            sem_clocks.pop(sid, None)
                            continue
                        c0 = sem_count.get(sid, 0)
                        if c0 is None:
                            continue
                        inc = u.update_value
                        clk = dict(V)
                        if sid in my_dma_sems:
                            # completion clock inherits all prior same-queue
                            # completions (in-order queue)
                            clk[sid] = max(clk.get(sid, 0), c0)
                        for k in range(1, inc + 1):
                            ck = dict(clk)
                            ck[sid] = max(ck.get(sid, 0), c0 + k)
                            sem_clocks.setdefault(sid, {})[c0 + k] = ck
                        sem_count[sid] = c0 + inc
    return n


def _strip_overhead(nc, entry_barrier=True, sem_clear=True, merge_blocks=True):
    """Remove fixed-cost instructions that this program never relies on:
    - the per-engine preamble RegisterMoves (zero/bounds-check registers;
      no dynamic APs or bounds-checked DMAs exist here),
    - const-pool memsets whose memref no instruction reads,
    - the trailing all_engine_barrier after the semaphore clear (execution
      completion already implies all engines are done; the cleared
      semaphores are only read by the *next* invocation),
    - tail drains on engine sems (every engine instruction has a transitive
      consumer path to the final out-DMA, so engine sems are already at
      their final values; only the DMA-queue sem needs a drain),
    - optionally the entry all_engine_barrier (cross-engine data deps are
      fully expressed via Tile semaphores),
    - optionally the semaphore clear itself (valid only if the runtime
      resets semaphores between NEFF executions),
    and finally merges the basic blocks, dropping the per-engine
    UnconditionalBranch block terminators.
    """
    blocks = [bb for f in nc.m.functions for bb in f.blocks]
    main = blocks[0]
    # 1. preamble RegisterMoves live in the first block
    main.instructions = [
        ins for ins in main.instructions
        if not isinstance(ins, mybir.InstRegisterMove)
    ]
    # 2. unreferenced const memsets
    used_memrefs = set()
    for bb in blocks:
        for ins in bb.instructions:
            for ap in list(ins.ins or []):
                mr = getattr(ap, "memref", None)
                if mr is not None:
                    used_memrefs.add(str(mr))
    def _is_dead_const(ins):
        if not isinstance(ins, mybir.InstMemset):
            return False
        outs = list(ins.outs or [])
        if len(outs) != 1:
            return False
        mr = getattr(outs[0], "memref", None)
        return (mr is not None and str(mr).startswith("const")
                and str(mr) not in used_memrefs)
    main.instructions = [i for i in main.instructions if not _is_dead_const(i)]
    # 3. drop everything after the last InstISA (sem_clear) in the last
    #    block; with sem_clear=False also drop the clear and its barrier
    end = blocks[-1]
    isa_idx = max(
        (i for i, ins in enumerate(end.instructions)
         if isinstance(ins, mybir.InstISA)),
        default=None,
    )
    if isa_idx is not None:
        end.instructions = end.instructions[: isa_idx + 1]
    if not sem_clear:
        # keep only the leading tail-drain group (everything before the
        # first EventSemaphore)
        cut = next(
            (i for i, ins in enumerate(end.instructions)
             if isinstance(ins, mybir.InstEventSemaphore)),
            len(end.instructions),
        )
        end.instructions = [
            ins for ins in end.instructions[:cut]
            if not isinstance(ins, mybir.InstDrain)
            or "DMAHW" in _drain_wait_name(ins)
        ]
    # 4. tail drains on engine sems are redundant (see docstring); barrier
    #    drains (which update the gather sem) must stay
    else:
        def _removable(ins):
            if not isinstance(ins, mybir.InstDrain):
                return False
            si = ins.sync_info
            if si is not None and si.on_update:
                return False
            nm = _drain_wait_name(ins)
            return "DMAHW" not in nm and "barrier" not in nm
        end.instructions = [i for i in end.instructions if not _removable(i)]
    # 5. optional: entry barrier
    if not entry_barrier:
        main.instructions = [
            ins for ins in main.instructions
            if not isinstance(ins, (mybir.InstDrain, mybir.InstEventSemaphore))
        ]
    # 6. merge blocks, dropping UnconditionalBranch terminators
    if merge_blocks and len(blocks) > 1:
        allins = []
        for bb in blocks:
            allins.extend(
                i for i in bb.instructions
                if not isinstance(i, mybir.InstUnconditionalBranch)
            )
        main.instructions = allins
        for f in nc.m.functions:
            f.blocks = [bb for bb in f.blocks if bb is main]


def _drain_wait_name(ins):
    si = ins.sync_info
    if si is None or not si.on_wait:
        return ""
    return " ".join(str(getattr(w, "ant_name", "")) for w in si.on_wait)


def resize_weight_diagonals():
    """Replicate jax.image.resize(..., 'bilinear') 129->128 (antialias=True).

    Returns (w_lo[k], w_hi[k]) with out[i] = w_lo[i]*y[i] + w_hi[i]*y[i+1].
    """
    in_size, out_size = H + 1, H
    scale = out_size / in_size
    kernel_scale = max(1.0 / scale, 1.0)
    sample_f = ((np.arange(out_size, dtype=np.float32) + 0.5) / scale - 0.5)
    x = np.abs(sample_f[None, :] - np.arange(in_size, dtype=np.float32)[:, None])
    x = x / kernel_scale
    wmat = np.clip(1.0 - x, 0.0, None).astype(np.float32)  # [in, out]
    total = wmat.sum(axis=0, keepdims=True)
    wmat = np.where(np.abs(total) > 1e-6, wmat / total, 0.0).astype(np.float32)
    lo = np.array([wmat[i, i] for i in range(out_size)], np.float32)
    hi = np.array([wmat[i + 1, i] for i in range(out_size)], np.float32)
    chk = np.zeros_like(wmat)
    for i in range(out_size):
        chk[i, i] = lo[i]
        chk[i + 1, i] = hi[i]
    assert np.allclose(chk, wmat, atol=1e-6), "resize weights not 2-tap"
    return lo, hi


@functools.lru_cache(maxsize=4)
def build_program(repeats=1, split=True, phase=99):
    _patch_tile_drain()
    _apply_queue_opt()
    nc = bass.Bass()

    x1b = nc.dram_tensor("x1b", [128, XHALF], BF, kind="ExternalInput")
    # x2 arrives host-padded to 3712 pixels with 1.0 fill (keeps the padded
    # pixels' norms finite without any device-side memset)
    x2c = nc.dram_tensor("x2c", [C, PIX2P], BF, kind="ExternalInput")
    # uv arrives host-transposed as [p, q, i] so the DMA is one contiguous
    # 2KB chunk per partition
    uv = nc.dram_tensor("uv", [128, 4, 128], F32, kind="ExternalInput")
    out = nc.dram_tensor("out", [128, 128], F32, kind="ExternalOutput")

    ZBIAS = [None]

    def zbias(p):
        return bass.AP(tensor=ZBIAS[0].tensor, offset=ZBIAS[0].offset,
                       ap=[[KO * RP, p], [1, 1]])

    with tile.TileContext(nc) as tc:
        with tc.tile_pool(name="sb", bufs=1) as sb:
            for _rep in range(repeats):
                try:
                    # ZTp doubles as the zero-bias source for activations
                    # (cols 0:3 of every row stay zero); memset comes first
                    # so bias reads see initialized zeros
                    ZTp = sb.tile([128, KO, RP], F32)
                    nc.vector.memset(ZTp[:, :, :], 0.0)
                    ZBIAS[0] = ZTp

                    # ---------- persistent SBUF ----------
                    sx1a = sb.tile([64, XHALF], BF)
                    nc.sync.dma_start(sx1a[:, :], x1b[0:64, :])
                    sx1b = sb.tile([64, XHALF], BF)
                    nc.sync.dma_start(sx1b[:, :], x1b[64:128, :])
                    sx1h = (sx1a, sx1b)
                    sUV = sb.tile([128, 4, 128], F32)
                    nc.sync.dma_start(sUV[:, :, :], uv[:, :, :])

                    # ---------- x2 -> Kb ----------
                    Kb = sb.tile([64, KOP], BF)
                    nc.vector.memset(Kb[:, KO:KOP], 0.0)
                    sx2 = sb.tile([C, PIX2P], BF)
                    nc.sync.dma_start(sx2[:, :], x2c[:, :])
                    # one [128, 29+64+64, 64] transposed-RAW buffer (x2 slab
                    # 0:29, x1 halves 29:93 / 93:157): transpose first, then
                    # ONE square-multiply + ONE reduce + ONE sqrt + ONE
                    # reciprocal covers every pixel norm in the problem
                    nsqraw = sb.tile([128, 157, C], BF)
                    nc.sync.dma_start_transpose(nsqraw[:, 0:29, :], sx2[:, :])
                    nc.sync.dma_start_transpose(nsqraw[:, 29:93, :], sx1a[:, :])
                    nc.sync.dma_start_transpose(nsqraw[:, 93:157, :], sx1b[:, :])
                    nsqsq = sb.tile([128, 157, C], BF)
                    nc.vector.tensor_mul(nsqsq[:, :, :], nsqraw[:, :, :],
                                         nsqraw[:, :, :])
                    nsq = sb.tile([128, 157], F32)
                    nc.vector.tensor_reduce(nsq[:, :], nsqsq[:, :, :],
                                            axis=mybir.AxisListType.X,
                                            op=mybir.AluOpType.add)
                    nss = sb.tile([128, 157], F32)
                    nc.scalar.activation(nss[:, :], nsq[:, :],
                                         mybir.ActivationFunctionType.Sqrt,
                                         bias=zbias(128))
                    nra = sb.tile([128, 157], F32)
                    nc.vector.reciprocal(nra[:, :], nss[:, :])
                    s2r = nra[:, 0:29]        # x2 pixel 1/norm [p, blk]
                    # nrmT = nra[:, 29:157]   # x1 1/norm [j, r]

                    dr_cm = tc.tile_pool(name="x2dram", bufs=1, space="DRAM")
                    dr = dr_cm.__enter__()
                    s2rd = dr.tile([128, 29], F32)
                    nc.sync.dma_start(s2rd[:, :], s2r)
                    # broadcast DRAM->SBUF replicated over the 64 channel
                    # partitions, kept in [c, p, blk] order (contiguous DMAs)
                    nrm2bcP = sb.tile([C, 128, 29], F32)
                    bc_src = bass.AP(tensor=s2rd.tensor, offset=s2rd.offset,
                                     ap=[[0, C], [29, 128], [1, 29]])
                    nc.sync.dma_start(nrm2bcP[:, :, :], bc_src)
                    dr_cm.__exit__(None, None, None)
                    # single normalize multiply over all 29 blocks
                    # (pix = blk*128 + p; tail pixels become 0*inf = NaN but
                    # are never read by the fold below)
                    x2n = sb.tile([C, PIX2P], BF)
                    dst_v = bass.AP(tensor=x2n.tensor, offset=x2n.offset,
                                    ap=[[PIX2P, C], [128, 29], [1, 128]])
                    in0_v = bass.AP(tensor=sx2.tensor, offset=sx2.offset,
                                    ap=[[PIX2P, C], [128, 29], [1, 128]])
                    in1_v = bass.AP(tensor=nrm2bcP.tensor, offset=nrm2bcP.offset,
                                    ap=[[128 * 29, C], [1, 29], [29, 128]])
                    nc.vector.tensor_mul(dst_v, in0_v, in1_v)
                    # fold both grid axes at once, straight into bf16 Kb:
                    # Kb[c, (dy,dx)] = sum_{i0,i1} x2n[c, (i0*6+dy)*60 + i1*6+dx]
                    in_f = bass.AP(tensor=x2n.tensor, offset=x2n.offset,
                                   ap=[[PIX2P, C], [w2, PS], [1, PS],
                                       [PS * w2, 10], [PS, 10]])
                    with nc.allow_low_precision(
                            reason="final bf16 round only; accumulation is f32"):
                        nc.vector.tensor_reduce(Kb[:, 0:KO], in_f,
                                                axis=mybir.AxisListType.XY,
                                                op=mybir.AluOpType.add)

                    if phase <= 1:
                        raise _PhaseStop(Kb[:, 0:KO])
                    if phase <= 2:
                        raise _PhaseStop(nra[:, 29:157])

                    # ---------- conv: Kb stationary, 32 streamed matmuls ----
                    Zsb = sb.tile([KOP, HWPIX], BF)
                    with tc.tile_pool(name="psc", bufs=1, space="PSUM") as psc:
                        for t4 in range(4):
                            pz = psc.tile([KOP, 4096], F32, tag="zz")
                            for c4 in range(8):
                                k = t4 * 8 + c4
                                hh = k // 16
                                nn = k % 16
                                nc.tensor.matmul(
                                    pz[:, 512 * c4:512 * (c4 + 1)],
                                    Kb[:, :],
                                    sx1h[hh][:, 512 * nn:512 * (nn + 1)],
                                    start=True, stop=True)
                            nc.vector.tensor_copy(
                                Zsb[:, 4096 * t4:4096 * (t4 + 1)], pz[:, :])

                    # HW/CoreSim xbar semantics: ZTt[j, r, o] = Z[o, r*128+j]
                    ZTt = sb.tile([128, 128, KOP], BF)
                    if OPT["big_xbar"]:
                        nc.sync.dma_start_transpose(ZTt[:, :, :], Zsb[:, :])
                    else:
                        for q in range(4):
                            nc.sync.dma_start_transpose(
                                ZTt[:, 32 * q:32 * (q + 1), :],
                                Zsb[:, 4096 * q:4096 * (q + 1)])

                    if phase == 25:
                        dbg = sb.tile([KOP, 128], F32)
                        nc.vector.tensor_copy(dbg[:, :], Zsb[:, 0:128])
                        raise _PhaseStop(dbg)

                    # ---------- normalize into zero-padded [j, o, rpad] -----
                    # (ZTp allocated + zeroed at the top of the pipeline)
                    zt_in = bass.AP(tensor=ZTt.tensor, offset=ZTt.offset,
                                    ap=[[128 * KOP, 128], [1, KO], [KOP, 128]])
                    # nrmT[j, r] lives at nra[:, 29:157]
                    nrm_bc = bass.AP(tensor=nra.tensor, offset=nra.offset + 29,
                                     ap=[[157, 128], [0, KO], [1, 128]])
                    zp_dst = bass.AP(tensor=ZTp.tensor, offset=ZTp.offset + 3,
                                     ap=[[KO * RP, 128], [RP, KO], [1, 128]])
                    nc.vector.tensor_mul(zp_dst, zt_in, nrm_bc)

                    if phase <= 3:
                        raise _PhaseStop(ZTp[:, 0, 3:131])

                    # ---------- col2im dy-fold: one strided reduce ----------
                    # Ydx[j, dx, I] = sum_dy ZTp[j, (dy*6+dx), I + dy]
                    # free offset = dx*134 + I + dy*(6*134 + 1)
                    Ydx = sb.tile([128, PS, IP], F32)
                    ydx_in = bass.AP(tensor=ZTp.tensor, offset=ZTp.offset,
                                     ap=[[KO * RP, 128], [RP, PS], [1, IP],
                                         [PS * RP + 1, PS]])
                    nc.vector.tensor_reduce(Ydx[:, :, :], ydx_in,
                                            axis=mybir.AxisListType.X,
                                            op=mybir.AluOpType.add)

                    if phase <= 4:
                        raise _PhaseStop(Ydx[:, 0, 0:128])

                    # ---------- I-resize (2-tap) in free dim -> bf16 --------
                    V0 = bass.AP(tensor=sUV.tensor, offset=sUV.offset + 2 * 128,
                                 ap=[[4 * 128, 128], [0, PS], [1, 128]])
                    V1 = bass.AP(tensor=sUV.tensor, offset=sUV.offset + 3 * 128,
                                 ap=[[4 * 128, 128], [0, PS], [1, 128]])
                    ti0 = sb.tile([128, PS, 128], F32)
                    ti1 = sb.tile([128, PS, 128], F32)
                    nc.vector.tensor_mul(ti0[:, :, :], Ydx[:, :, 0:128], V0)
                    nc.vector.tensor_mul(ti1[:, :, :], Ydx[:, :, 1:129], V1)
                    tI = sb.tile([128, PS, 128], BF)
                    nc.vector.tensor_add(tI[:, :, :], ti0[:, :, :], ti1[:, :, :])

                    # ---------- xbar to contiguous, then pad-copy -----------
                    # T2c[Io, dx, j] = tI[j, dx, Io]  (HW xbar needs a
                    # contiguous dest free layout; a strided window dest
                    # silently lands elsewhere -- found the hard way)
                    T2c = sb.tile([128, PS, 128], BF)
                    nc.sync.dma_start_transpose(T2c[:, :, :], tI[:, :, :])
                    T2p = sb.tile([128, PS, RP], BF)
                    nc.vector.memset(T2p[:, :, :], 0.0)
                    nc.vector.tensor_copy(T2p[:, :, 3:131], T2c[:, :, :])

                    if phase <= 5:
                        raise _PhaseStop(T2p[:, 0, 3:131])

                    # ---------- col2im dx-fold: one strided reduce ----------
                    # Y2[Io, J] = sum_dx T2p[Io, dx, J + dx]
                    # free offset = dx*134 + J + dx = dx*135 + J
                    Y2 = sb.tile([128, IP], F32)
                    y2_in = bass.AP(tensor=T2p.tensor, offset=T2p.offset,
                                    ap=[[PS * RP, 128], [1, IP], [RP + 1, PS]])
                    nc.vector.tensor_reduce(Y2[:, :], y2_in,
                                            axis=mybir.AxisListType.X,
                                            op=mybir.AluOpType.add)

                    if phase <= 6:
                        raise _PhaseStop(Y2[:, 0:128])

                    # ---------- J-resize (2-tap, scaled) -> out [I, J] ------
                    U0 = bass.AP(tensor=sUV.tensor, offset=sUV.offset,
                                 ap=[[4 * 128, 128], [1, 128]])
                    U1 = bass.AP(tensor=sUV.tensor, offset=sUV.offset + 128,
                                 ap=[[4 * 128, 128], [1, 128]])
                    o0 = sb.tile([128, 128], F32)
                    o1 = sb.tile([128, 128], F32)
                    nc.vector.tensor_mul(o0[:, :], Y2[:, 0:128], U0)
                    nc.vector.tensor_mul(o1[:, :], Y2[:, 1:129], U1)
                    osb = sb.tile([128, 128], F32)
                    nc.vector.tensor_add(osb[:, :], o0[:, :], o1[:, :])
                    nc.sync.dma_start(out[:, :], osb[:, :])
                except _PhaseStop as e:
                    pp, ff = e.ap.shape[0], int(np.prod(e.ap.shape[1:]))
                    dap = e.ap
                    if dap.dtype != F32:
                        cvt = sb.tile([pp, ff], F32)
                        nc.vector.tensor_copy(cvt[:, :], dap)
                        dap = cvt[:, :]
                    nc.sync.dma_start(out[0:pp, 0:ff], dap)

    if OPT["dedup_ldw"]:
        _dedup_ldweights(nc)
    if OPT["reduce_waits"]:
        _reduce_waits(nc)
    if OPT["strip"]:
        _strip_overhead(nc, entry_barrier=OPT["entry_barrier"],
                        merge_blocks=OPT["merge_blocks"])
    if split:
        _split_excess_waits(nc)
    return nc


@functools.lru_cache(maxsize=1)
def _host_constants():
    lo, hi = resize_weight_diagonals()
    u0 = (lo * SCALE).astype(np.float32)
    u1 = (hi * SCALE).astype(np.float32)
    uv = np.stack([
        np.tile(u0[None, :], (128, 1)),
        np.tile(u1[None, :], (128, 1)),
        np.tile(lo[None, :], (128, 1)),
        np.tile(hi[None, :], (128, 1)),
    ]).astype(np.float32)
    # [q, p, i] -> [p, q, i]: contiguous per-partition layout for the DMA
    return np.ascontiguousarray(uv.transpose(1, 0, 2))


def make_in_maps(x1, x2):
    x1 = np.asarray(x1)
    x2 = np.asarray(x2)
    uv = _host_constants()
    x1bf = x1.astype(BF16)
    in_maps = []
    for b in range(B):
        xb = x1bf[b]  # [64, 128, 128]
        x1arr = np.concatenate(
            [xb[:, :64].reshape(C, XHALF), xb[:, 64:].reshape(C, XHALF)], axis=0
        )  # [128, 8192]
        x2p = np.concatenate(
            [x2[b].reshape(C, PIX2), np.ones((C, PIX2P - PIX2), np.float32)],
            axis=1,
        ).astype(BF16)
        in_maps.append({
            "x1b": np.ascontiguousarray(x1arr),
            "x2c": np.ascontiguousarray(x2p),
            "uv": uv,
        })
    return in_maps


def kernel(x1, x2):
    from concourse.bass_utils import run_bass_kernel_spmd

    nc = build_program()
    in_maps = make_in_maps(x1, x2)
    res = run_bass_kernel_spmd(nc, in_maps, core_ids=list(range(NCORES)))
    outs = [np.asarray(res.results[b]["out"]) for b in range(B)]  # [I, J]
    return np.stack(outs)[:, None].astype(np.float32)


# revision 53
# speedup vs baseline: 2.1516x; 2.1516x over previous
"""Trainium2 Bass kernel for nn_CrossCorrV2.

Math: with P = nd0*nd1 = 100 patches and OUT_CHANNEL = 100, top_k over the
patch axis returns *all* patches, so mean(top_k) == mean over patches.  Both
the grouped conv (linear in the filter) and the bilinear resize (linear map)
commute with that mean, so the whole module collapses per sample to

    out[b] = resize129->128( corr2d(x1n[b], K[b]) ) / (hh*ww*P)

where x1n/x2n are channel-L2-normalized and K[b][c, dy, dx] =
sum_{grid} x2n[b, c, i0*6+dy, i1*6+dx] (a 6x6 fold of the normalized x2).

Per-core pipeline (1 sample per NeuronCore, 8 cores).  This environment
executes NEFFs at a fixed ~20-40us per *instruction*, so the kernel is
instruction-count optimized: every fold/shift is expressed as a single
strided-AP instruction and all PE transposes are replaced by xbar DMA
transposes plus free-dim reductions.

  - x2 ([c, pix]): square -> xbar -> reduce -> sqrt -> recip -> DRAM
    broadcast -> one normalize multiply -> ONE nested-AP tensor_reduce
    (both grid axes at once) -> Kb [64, 48] bf16.
  - x1 norms: two squares (ACT/DVE) -> two xbar transposes -> two free
    reduces -> sqrt -> recip -> nrmT [j, r].
  - conv: Kb stationary, 32 matmuls streaming x1 [64, 512] chunks ->
    Z [48, 16384] PSUM -> 4 copy-casts to bf16 -> ONE xbar transpose ->
    ZTt [j, r, o] -> one broadcast multiply by nrmT writing into a
    zero-padded [j, o, rpad=134] layout.
  - col2im dy-fold: ONE tensor_reduce with a stride-805 virtual axis
    (805 = 6*134 + 1 couples the o-step and the r-shift).
  - I-resize (2-tap) in free dim: 3 vector ops -> bf16 [j, dx, Io].
  - ONE xbar transpose into zero-padded [Io, dx, jpad=134]; col2im
    dx-fold: ONE tensor_reduce with a stride-135 virtual axis.
  - J-resize in free dim: 3 vector ops -> out [I, J] row-major, 1 DMA.

A post-legalization pass dedups the per-matmul InstLdweights that tile
legalization inserts (the PE array retains stationary weights across
matmuls), collapsing 32 identical weight loads into 1.
"""

import os
import sys
import functools

import numpy as np

for _p in ("/opt/trn_rl_repo", "/root/.axon_site/_ro/trn_rl_repo"):
    if os.path.isdir(_p) and _p not in sys.path:
        sys.path.insert(0, _p)

import ml_dtypes
import concourse.bass as bass
import concourse.mybir as mybir
import concourse.tile as tile
import concourse.tile_sem_assignment as tsa
from concourse.vector_clock import ScopedClock

# Optimization switches (module-level so a bisect harness can flip them;
# the lru_cache on build_program must be cleared after changing any).
OPT = {
    "one_queue": True,     # collapse HW-DGE queue round-robin to 1 queue
    "dedup_ldw": True,     # drop repeated identical InstLdweights
    "reduce_waits": True,  # transitive reduction of semaphore waits
    "strip": True,         # preamble regs / dead consts / barrier tail
    "entry_barrier": False,
    "merge_blocks": True,
    "big_xbar": True,      # one 16384-wide Z transpose instead of 4
    "sem_clear": False,    # rely on the runtime resetting sems per execution
}

# Collapse the HW-DGE queue round-robin to one queue: every DMA then ticks
# the same DMAHW0 semaphore, so the TileContext tail drain needs 1 wait
# instead of 8 (each wait is a whole Drain instruction in this walrus).
_ORIG_NUM_HWDGE = tsa.NUM_HWDGE_SEMS


def _apply_queue_opt():
    nq = OPT.get("nqueues", 1 if OPT["one_queue"] else _ORIG_NUM_HWDGE)
    tsa.NUM_HWDGE_SEMS = nq

BF16 = ml_dtypes.bfloat16
F32 = mybir.dt.float32
BF = mybir.dt.bfloat16

B, C, H, W = 8, 64, 128, 128
h2, w2 = 60, 60
PS = 6                      # patch size (hh == ww == 6)
KO = PS * PS                # 36 filter taps
KOP = 48                    # padded taps (xbar transpose needs mult of 16)
NPATCH = 100
SCALE = 1.0 / (PS * PS * NPATCH)   # 1/3600
NCORES = 8

HWPIX = H * W               # 16384
XHALF = HWPIX // 2          # 8192
PIX2 = h2 * w2              # 3600
PIX2P = 29 * 128            # 3712 (x2 pixels padded to xbar granule)
RP = H + PS                 # 134: padded r pitch (129 outputs + 5 slack)
IP = H + 1                  # 129 conv output rows/cols


class _PhaseStop(Exception):
    def __init__(self, ap):
        self.ap = ap


def _patch_tile_drain():
    """Replace the TileContext tail (drain + 2 all-engine barriers + sem
    clear) with the minimal equivalent: ONE Pool no-op carrying the final-
    clock waits for every semaphore (split later into 1-wait no-ops by
    _split_excess_waits), then the DGE reset + semaphore clear on Pool.
    Engine streams quiesce transitively (every instruction has a consumer
    path into the final waits), so no gather/release barrier is needed."""
    if getattr(tile.TileContext, "_drain_patched", False):
        return

    def _patched(self, tick_clock, wait_clock):
        nc = self.nc
        nop = mybir.InstNoOp(
            name="tail_wait", engine=mybir.EngineType.Pool, ins=[], outs=[]
        )
        nop_bi = nc.gpsimd.add_instruction(nop)
        wait_clock.add_sem_waits(
            nop_bi.ins, ScopedClock({None: tick_clock.global_clock})
        )
        popped = nc._tile_sem_poison_stack.pop()
        assert popped is self._sem_poison
        if OPT["sem_clear"]:
            nc.clear_and_free_semaphores(list(self.sems.allocated().values()))

    tile.TileContext._drain_and_barrier = _patched
    tile.TileContext._drain_patched = True


def _split_excess_waits(nc):
    """Walrus here rejects >1 sync wait per instruction; move excess waits
    onto same-engine NoOps spliced immediately before the instruction."""
    n = 0
    for f in nc.m.functions:
        for bb in f.blocks:
            out = []
            for ins in bb.instructions:
                si = ins.sync_info
                if si is not None and si.on_wait and len(si.on_wait) > 1:
                    waits = list(si.on_wait)
                    for j, w in enumerate(waits[:-1]):
                        nop = mybir.InstNoOp(
                            name=f"{ins.name}_sw{j}",
                            engine=ins.engine,
                            ins=[],
                            outs=[],
                            sync_info=mybir.SyncInfo(on_wait=[w], on_update=[]),
                        )
                        out.append(nop)
                        n += 1
                    ins.sync_info = mybir.SyncInfo(
                        on_wait=[waits[-1]], on_update=list(si.on_update or [])
                    )
                out.append(ins)
            bb.instructions = out
    return n


def _dedup_ldweights(nc):
    """Tile legalization pairs every InstMatmult with an InstLdweights even
    when the stationary weights are unchanged.  The PE array retains loaded
    weights across matmuls, so drop consecutive identical loads, folding the
    removed load's sync_info into the following instruction (the matmult).
    Run BEFORE _split_excess_waits so merged waits get re-split if needed."""
    n = 0
    for f in nc.m.functions:
        for bb in f.blocks:
            out = []
            last_sig = None
            for ins in bb.instructions:
                if isinstance(ins, mybir.InstLdweights):
                    sig = (
                        str(ins.ins),
                        getattr(ins, "is_transpose", None),
                        str(getattr(ins, "tile_position", None)),
                        str(getattr(ins, "tile_size", None)),
                        getattr(ins, "perf_mode", None),
                    )
                    if sig == last_sig:
                        # redundant load: push its sync onto the next inst
                        si = ins.sync_info
                        if si is not None and (si.on_wait or si.on_update):
                            ins._pending_sync = si  # stash
                            out.append(("PENDING", si))
                        n += 1
                        continue
                    last_sig = sig
                    out.append(ins)
                else:
                    if isinstance(ins, mybir.InstMatmult) and getattr(
                        ins, "is_transpose", False
                    ):
                        last_sig = None  # transposes clobber the array
                    out.append(ins)
            # fold pending syncs into the next real instruction
            merged = []
            pend_waits, pend_upds = [], []
            for item in out:
                if isinstance(item, tuple) and item[0] == "PENDING":
                    si = item[1]
                    pend_waits.extend(list(si.on_wait or []))
                    pend_upds.extend(list(si.on_update or []))
                    continue
                ins = item
                if pend_waits or pend_upds:
                    si = ins.sync_info
                    w = list(si.on_wait or []) if si else []
                    u = list(si.on_update or []) if si else []
                    ins.sync_info = mybir.SyncInfo(
                        on_wait=pend_waits + w, on_update=pend_upds + u
                    )
                    pend_waits, pend_upds = [], []
                merged.append(ins)
            assert not pend_waits and not pend_upds
            bb.instructions = merged
    return n


_INC_MODES = ("sem-inc", "sem-add-imm")


def _reduce_waits(nc):
    """Transitive reduction of semaphore waits.  A wait (S >= v) is dropped
    when the waiting engine's happens-before clock already covers it —
    through program order on the engine plus the clocks carried by
    previously-joined waits.  Sound because sems are monotone counters and
    the scheduled instruction order lists every (S, v) producer before any
    consumer referencing it."""
    engine_clock = {}
    sem_count = {}
    sem_clocks = {}
    n = 0
    dma_types = (mybir.InstDMACopy, mybir.InstDmaTransposeAnt)
    for f in nc.m.functions:
        for bb in f.blocks:
            for ins in bb.instructions:
                V = engine_clock.setdefault(ins.engine, {})
                is_dma = isinstance(ins, dma_types)
                my_dma_sems = set()
                si = ins.sync_info
                if si is not None and is_dma and si.on_update:
                    my_dma_sems = {
                        u.id for u in si.on_update
                        if getattr(u, "update_mode", "") in _INC_MODES
                    }
                keep_all = OPT["sem_clear"] and (
                    ins.name == "tail_wait"
                    or ins.name.startswith("tail_wait_sw"))
                if si is not None and si.on_wait:
                    # fixpoint: a wait is redundant if covered by the engine
                    # clock plus the producer clocks of the OTHER kept waits
                    ge, keep_other = [], []
                    for w in si.on_wait:
                        (ge if getattr(w, "wait_mode", "") == "sem-ge-imm"
                         else keep_other).append(w)
                    def _pclock(w):
                        pc = dict(sem_clocks.get(w.id, {}).get(w.wait_value, {}))
                        pc[w.id] = max(pc.get(w.id, 0), w.wait_value)
                        return pc
                    kept = list(ge)
                    if not keep_all:
                        changed = True
                        while changed:
                            changed = False
                            for w in list(kept):
                                cov = dict(V)
                                for w2 in kept:
                                    if w2 is w:
                                        continue
                                    for s2, v2 in _pclock(w2).items():
                                        if cov.get(s2, 0) < v2:
                                            cov[s2] = v2
                                implied = cov.get(w.id, 0) >= w.wait_value
                                if implied:
                                    kept.remove(w)
                                    n += 1
                                    changed = True
                                    break
                    for w in kept:
                        for s2, v2 in _pclock(w).items():
                            if V.get(s2, 0) < v2:
                                V[s2] = v2
                    if keep_all:
                        # still record coverage for downstream bookkeeping
                        pass
                    if len(kept) + len(keep_other) != len(si.on_wait):
                        ins.sync_info = mybir.SyncInfo(
                            on_wait=keep_other + kept,
                            on_update=list(si.on_update or []),
                        )
                si = ins.sync_info
                if si is not None and si.on_update:
                    for u in si.on_update:
                        sid = u.id
                        if getattr(u, "update_mode", "") not in _INC_MODES:
                            sem_count[sid] = None  # opaque: stop tracking
                            sem_clocks.pop(sid, None)
                            continue
                        c0 = sem_count.get(sid, 0)
                        if c0 is None:
                            continue
                        inc = u.update_value
                        clk = dict(V)
                        if sid in my_dma_sems:
                            # completion clock inherits all prior same-queue
                            # completions (in-order queue)
                            clk[sid] = max(clk.get(sid, 0), c0)
                        for k in range(1, inc + 1):
                            ck = dict(clk)
                            ck[sid] = max(ck.get(sid, 0), c0 + k)
                            sem_clocks.setdefault(sid, {})[c0 + k] = ck
                        sem_count[sid] = c0 + inc
    return n


def _strip_overhead(nc, entry_barrier=True, sem_clear=True, merge_blocks=True):
    """Remove fixed-cost instructions that this program never relies on:
    - the per-engine preamble RegisterMoves (zero/bounds-check registers;
      no dynamic APs or bounds-checked DMAs exist here),
    - const-pool memsets whose memref no instruction reads,
    - the trailing all_engine_barrier after the semaphore clear (execution
      completion already implies all engines are done; the cleared
      semaphores are only read by the *next* invocation),
    - tail drains on engine sems (every engine instruction has a transitive
      consumer path to the final out-DMA, so engine sems are already at
      their final values; only the DMA-queue sem needs a drain),
    - optionally the entry all_engine_barrier (cross-engine data deps are
      fully expressed via Tile semaphores),
    - optionally the semaphore clear itself (valid only if the runtime
      resets semaphores between NEFF executions),
    and finally merges the basic blocks, dropping the per-engine
    UnconditionalBranch block terminators.
    """
    blocks = [bb for f in nc.m.functions for bb in f.blocks]
    main = blocks[0]
    # 1. preamble RegisterMoves live in the first block
    main.instructions = [
        ins for ins in main.instructions
        if not isinstance(ins, mybir.InstRegisterMove)
    ]
    # 2. unreferenced const memsets
    used_memrefs = set()
    for bb in blocks:
        for ins in bb.instructions:
            for ap in list(ins.ins or []):
                mr = getattr(ap, "memref", None)
                if mr is not None:
                    used_memrefs.add(str(mr))
    def _is_dead_const(ins):
        if not isinstance(ins, mybir.InstMemset):
            return False
        outs = list(ins.outs or [])
        if len(outs) != 1:
            return False
        mr = getattr(outs[0], "memref", None)
        return (mr is not None and str(mr).startswith("const")
                and str(mr) not in used_memrefs)
    main.instructions = [i for i in main.instructions if not _is_dead_const(i)]
    # 3. drop everything after the last InstISA (sem_clear) in the last
    #    block; with sem_clear=False also drop the clear and its barrier
    end = blocks[-1]
    isa_idx = max(
        (i for i, ins in enumerate(end.instructions)
         if isinstance(ins, mybir.InstISA)),
        default=None,
    )
    if isa_idx is not None:
        end.instructions = end.instructions[: isa_idx + 1]
    if not sem_clear:
        # keep only the leading tail-drain group (everything before the
        # first EventSemaphore)
        cut = next(
            (i for i, ins in enumerate(end.instructions)
             if isinstance(ins, mybir.InstEventSemaphore)),
            len(end.instructions),
        )
        end.instructions = [
            ins for ins in end.instructions[:cut]
            if not isinstance(ins, mybir.InstDrain)
            or "DMAHW" in _drain_wait_name(ins)
        ]
    # 4. tail drains on engine sems are redundant (see docstring); barrier
    #    drains (which update the gather sem) must stay
    else:
        def _removable(ins):
            if not isinstance(ins, mybir.InstDrain):
                return False
            si = ins.sync_info
            if si is not None and si.on_update:
                return False
            nm = _drain_wait_name(ins)
            return "DMAHW" not in nm and "barrier" not in nm
        end.instructions = [i for i in end.instructions if not _removable(i)]
    # 5. optional: entry barrier
    if not entry_barrier:
        main.instructions = [
            ins for ins in main.instructions
            if not isinstance(ins, (mybir.InstDrain, mybir.InstEventSemaphore))
        ]
    # 6. merge blocks, dropping UnconditionalBranch terminators
    if merge_blocks and len(blocks) > 1:
        allins = []
        for bb in blocks:
            allins.extend(
                i for i in bb.instructions
                if not isinstance(i, mybir.InstUnconditionalBranch)
            )
        main.instructions = allins
        for f in nc.m.functions:
            f.blocks = [bb for bb in f.blocks if bb is main]


def _drain_wait_name(ins):
    si = ins.sync_info
    if si is None or not si.on_wait:
        return ""
    return " ".join(str(getattr(w, "ant_name", "")) for w in si.on_wait)


def resize_weight_diagonals():
    """Replicate jax.image.resize(..., 'bilinear') 129->128 (antialias=True).

    Returns (w_lo[k], w_hi[k]) with out[i] = w_lo[i]*y[i] + w_hi[i]*y[i+1].
    """
    in_size, out_size = H + 1, H
    scale = out_size / in_size
    kernel_scale = max(1.0 / scale, 1.0)
    sample_f = ((np.arange(out_size, dtype=np.float32) + 0.5) / scale - 0.5)
    x = np.abs(sample_f[None, :] - np.arange(in_size, dtype=np.float32)[:, None])
    x = x / kernel_scale
    wmat = np.clip(1.0 - x, 0.0, None).astype(np.float32)  # [in, out]
    total = wmat.sum(axis=0, keepdims=True)
    wmat = np.where(np.abs(total) > 1e-6, wmat / total, 0.0).astype(np.float32)
    lo = np.array([wmat[i, i] for i in range(out_size)], np.float32)
    hi = np.array([wmat[i + 1, i] for i in range(out_size)], np.float32)
    chk = np.zeros_like(wmat)
    for i in range(out_size):
        chk[i, i] = lo[i]
        chk[i + 1, i] = hi[i]
    assert np.allclose(chk, wmat, atol=1e-6), "resize weights not 2-tap"
    return lo, hi


@functools.lru_cache(maxsize=4)
def build_program(repeats=1, split=True, phase=99):
    _patch_tile_drain()
    _apply_queue_opt()
    nc = bass.Bass()

    x1b = nc.dram_tensor("x1b", [128, XHALF], BF, kind="ExternalInput")
    # x2 arrives host-padded to 3712 pixels with 1.0 fill (keeps the padded
    # pixels' norms finite without any device-side memset)
    x2c = nc.dram_tensor("x2c", [C, PIX2P], BF, kind="ExternalInput")
    # uv arrives host-transposed as [p, q, i] so the DMA is one contiguous
    # 2KB chunk per partition
    uv = nc.dram_tensor("uv", [128, 4, 128], F32, kind="ExternalInput")
    out = nc.dram_tensor("out", [128, 128], F32, kind="ExternalOutput")

    ZBIAS = [None]

    def zbias(p):
        return bass.AP(tensor=ZBIAS[0].tensor, offset=ZBIAS[0].offset,
                       ap=[[KO * RP, p], [1, 1]])

    with tile.TileContext(nc) as tc:
        with tc.tile_pool(name="sb", bufs=1) as sb:
            for _rep in range(repeats):
                try:
                    # ZTp doubles as the zero-bias source for activations
                    # (cols 0:3 of every row stay zero); memset comes first
                    # so bias reads see initialized zeros
                    ZTp = sb.tile([128, KO, RP], F32)
                    nc.vector.memset(ZTp[:, :, :], 0.0)
                    ZBIAS[0] = ZTp

                    # ---------- persistent SBUF ----------
                    sx1a = sb.tile([64, XHALF], BF)
                    nc.sync.dma_start(sx1a[:, :], x1b[0:64, :])
                    sx1b = sb.tile([64, XHALF], BF)
                    nc.sync.dma_start(sx1b[:, :], x1b[64:128, :])
                    sx1h = (sx1a, sx1b)
                    sUV = sb.tile([128, 4, 128], F32)
                    nc.sync.dma_start(sUV[:, :, :], uv[:, :, :])

                    # ---------- x2 -> Kb ----------
                    Kb = sb.tile([64, KOP], BF)
                    nc.vector.memset(Kb[:, KO:KOP], 0.0)
                    sx2 = sb.tile([C, PIX2P], BF)
                    nc.sync.dma_start(sx2[:, :], x2c[:, :])
                    # one [128, 29+64+64, 64] transposed-RAW buffer (x2 slab
                    # 0:29, x1 halves 29:93 / 93:157): transpose first, then
                    # ONE square-multiply + ONE reduce + ONE sqrt + ONE
                    # reciprocal covers every pixel norm in the problem
                    nsqraw = sb.tile([128, 157, C], BF)
                    nc.sync.dma_start_transpose(nsqraw[:, 0:29, :], sx2[:, :])
                    nc.sync.dma_start_transpose(nsqraw[:, 29:93, :], sx1a[:, :])
                    nc.sync.dma_start_transpose(nsqraw[:, 93:157, :], sx1b[:, :])
                    nsqsq = sb.tile([128, 157, C], BF)
                    nc.vector.tensor_mul(nsqsq[:, :, :], nsqraw[:, :, :],
                                         nsqraw[:, :, :])
                    nsq = sb.tile([128, 157], F32)
                    nc.vector.tensor_reduce(nsq[:, :], nsqsq[:, :, :],
                                            axis=mybir.AxisListType.X,
                                            op=mybir.AluOpType.add)
                    nss = sb.tile([128, 157], F32)
                    nc.scalar.activation(nss[:, :], nsq[:, :],
                                         mybir.ActivationFunctionType.Sqrt,
                                         bias=zbias(128))
                    nra = sb.tile([128, 157], F32)
                    nc.vector.reciprocal(nra[:, :], nss[:, :])
                    s2r = nra[:, 0:29]        # x2 pixel 1/norm [p, blk]
                    # nrmT = nra[:, 29:157]   # x1 1/norm [j, r]

                    dr_cm = tc.tile_pool(name="x2dram", bufs=1, space="DRAM")
                    dr = dr_cm.__enter__()
                    s2rd = dr.tile([128, 29], F32)
                    nc.sync.dma_start(s2rd[:, :], s2r)
                    # broadcast DRAM->SBUF replicated over the 64 channel
                    # partitions, kept in [c, p, blk] order (contiguous DMAs)
                    nrm2bcP = sb.tile([C, 128, 29], F32)
                    bc_src = bass.AP(tensor=s2rd.tensor, offset=s2rd.offset,
                                     ap=[[0, C], [29, 128], [1, 29]])
                    nc.sync.dma_start(nrm2bcP[:, :, :], bc_src)
                    dr_cm.__exit__(None, None, None)
                    # single normalize multiply over all 29 blocks
                    # (pix = blk*128 + p; tail pixels become 0*inf = NaN but
                    # are never read by the fold below)
                    x2n = sb.tile([C, PIX2P], BF)
                    dst_v = bass.AP(tensor=x2n.tensor, offset=x2n.offset,
                                    ap=[[PIX2P, C], [128, 29], [1, 128]])
                    in0_v = bass.AP(tensor=sx2.tensor, offset=sx2.offset,
                                    ap=[[PIX2P, C], [128, 29], [1, 128]])
                    in1_v = bass.AP(tensor=nrm2bcP.tensor, offset=nrm2bcP.offset,
                                    ap=[[128 * 29, C], [1, 29], [29, 128]])
                    nc.vector.tensor_mul(dst_v, in0_v, in1_v)
                    # fold both grid axes at once, straight into bf16 Kb:
                    # Kb[c, (dy,dx)] = sum_{i0,i1} x2n[c, (i0*6+dy)*60 + i1*6+dx]
                    in_f = bass.AP(tensor=x2n.tensor, offset=x2n.offset,
                                   ap=[[PIX2P, C], [w2, PS], [1, PS],
                                       [PS * w2, 10], [PS, 10]])
                    with nc.allow_low_precision(
                            reason="final bf16 round only; accumulation is f32"):
                        nc.vector.tensor_reduce(Kb[:, 0:KO], in_f,
                                                axis=mybir.AxisListType.XY,
                                                op=mybir.AluOpType.add)

                    if phase <= 1:
                        raise _PhaseStop(Kb[:, 0:KO])
                    if phase <= 2:
                        raise _PhaseStop(nra[:, 29:157])

                    # ---------- conv: Kb stationary, 32 streamed matmuls ----
                    Zsb = sb.tile([KOP, HWPIX], BF)
                    with tc.tile_pool(name="psc", bufs=1, space="PSUM") as psc:
                        for t4 in range(4):
                            pz = psc.tile([KOP, 4096], F32, tag="zz")
                            for c4 in range(8):
                                k = t4 * 8 + c4
                                hh = k // 16
                                nn = k % 16
                                nc.tensor.matmul(
                                    pz[:, 512 * c4:512 * (c4 + 1)],
                                    Kb[:, :],
                                    sx1h[hh][:, 512 * nn:512 * (nn + 1)],
                                    start=True, stop=True)
                            nc.vector.tensor_copy(
                                Zsb[:, 4096 * t4:4096 * (t4 + 1)], pz[:, :])

                    # HW/CoreSim xbar semantics: ZTt[j, r, o] = Z[o, r*128+j]
                    ZTt = sb.tile([128, 128, KOP], BF)
                    if OPT["big_xbar"]:
                        nc.sync.dma_start_transpose(ZTt[:, :, :], Zsb[:, :])
                    else:
                        for q in range(4):
                            nc.sync.dma_start_transpose(
                                ZTt[:, 32 * q:32 * (q + 1), :],
                                Zsb[:, 4096 * q:4096 * (q + 1)])

                    if phase == 25:
                        dbg = sb.tile([KOP, 128], F32)
                        nc.vector.tensor_copy(dbg[:, :], Zsb[:, 0:128])
                        raise _PhaseStop(dbg)

                    # ---------- normalize into zero-padded [j, o, rpad] -----
                    # (ZTp allocated + zeroed at the top of the pipeline)
                    zt_in = bass.AP(tensor=ZTt.tensor, offset=ZTt.offset,
                                    ap=[[128 * KOP, 128], [1, KO], [KOP, 128]])
                    # nrmT[j, r] lives at nra[:, 29:157]
                    nrm_bc = bass.AP(tensor=nra.tensor, offset=nra.offset + 29,
                                     ap=[[157, 128], [0, KO], [1, 128]])
                    zp_dst = bass.AP(tensor=ZTp.tensor, offset=ZTp.offset + 3,
                                     ap=[[KO * RP, 128], [RP, KO], [1, 128]])
                    nc.vector.tensor_mul(zp_dst, zt_in, nrm_bc)

                    if phase <= 3:
                        raise _PhaseStop(ZTp[:, 0, 3:131])

                    # ---------- col2im dy-fold: one strided reduce ----------
                    # Ydx[j, dx, I] = sum_dy ZTp[j, (dy*6+dx), I + dy]
                    # free offset = dx*134 + I + dy*(6*134 + 1)
                    Ydx = sb.tile([128, PS, IP], F32)
                    ydx_in = bass.AP(tensor=ZTp.tensor, offset=ZTp.offset,
                                     ap=[[KO * RP, 128], [RP, PS], [1, IP],
                                         [PS * RP + 1, PS]])
                    nc.vector.tensor_reduce(Ydx[:, :, :], ydx_in,
                                            axis=mybir.AxisListType.X,
                                            op=mybir.AluOpType.add)

                    if phase <= 4:
                        raise _PhaseStop(Ydx[:, 0, 0:128])

                    # ---------- I-resize (2-tap) in free dim -> bf16 --------
                    V0 = bass.AP(tensor=sUV.tensor, offset=sUV.offset + 2 * 128,
                                 ap=[[4 * 128, 128], [0, PS], [1, 128]])
                    V1 = bass.AP(tensor=sUV.tensor, offset=sUV.offset + 3 * 128,
                                 ap=[[4 * 128, 128], [0, PS], [1, 128]])
                    ti0 = sb.tile([128, PS, 128], F32)
                    ti1 = sb.tile([128, PS, 128], F32)
                    nc.vector.tensor_mul(ti0[:, :, :], Ydx[:, :, 0:128], V0)
                    nc.vector.tensor_mul(ti1[:, :, :], Ydx[:, :, 1:129], V1)
                    tI = sb.tile([128, PS, 128], BF)
                    nc.vector.tensor_add(tI[:, :, :], ti0[:, :, :], ti1[:, :, :])

                    # ---------- xbar to contiguous, then pad-copy -----------
                    # T2c[Io, dx, j] = tI[j, dx, Io]  (HW xbar needs a
                    # contiguous dest free layout; a strided window dest
                    # silently lands elsewhere -- found the hard way)
                    T2c = sb.tile([128, PS, 128], BF)
                    nc.sync.dma_start_transpose(T2c[:, :, :], tI[:, :, :])
                    T2p = sb.tile([128, PS, RP], BF)
                    nc.vector.memset(T2p[:, :, :], 0.0)
                    nc.vector.tensor_copy(T2p[:, :, 3:131], T2c[:, :, :])

                    if phase <= 5:
                        raise _PhaseStop(T2p[:, 0, 3:131])

                    # ---------- col2im dx-fold: one strided reduce ----------
                    # Y2[Io, J] = sum_dx T2p[Io, dx, J + dx]
                    # free offset = dx*134 + J + dx = dx*135 + J
                    Y2 = sb.tile([128, IP], F32)
                    y2_in = bass.AP(tensor=T2p.tensor, offset=T2p.offset,
                                    ap=[[PS * RP, 128], [1, IP], [RP + 1, PS]])
                    nc.vector.tensor_reduce(Y2[:, :], y2_in,
                                            axis=mybir.AxisListType.X,
                                            op=mybir.AluOpType.add)

                    if phase <= 6:
                        raise _PhaseStop(Y2[:, 0:128])

                    # ---------- J-resize (2-tap, scaled) -> out [I, J] ------
                    U0 = bass.AP(tensor=sUV.tensor, offset=sUV.offset,
                                 ap=[[4 * 128, 128], [1, 128]])
                    U1 = bass.AP(tensor=sUV.tensor, offset=sUV.offset + 128,
                                 ap=[[4 * 128, 128], [1, 128]])
                    o0 = sb.tile([128, 128], F32)
                    o1 = sb.tile([128, 128], F32)
                    nc.vector.tensor_mul(o0[:, :], Y2[:, 0:128], U0)
                    nc.vector.tensor_mul(o1[:, :], Y2[:, 1:129], U1)
                    osb = sb.tile([128, 128], F32)
                    nc.vector.tensor_add(osb[:, :], o0[:, :], o1[:, :])
                    nc.sync.dma_start(out[:, :], osb[:, :])
                except _PhaseStop as e:
                    pp, ff = e.ap.shape[0], int(np.prod(e.ap.shape[1:]))
                    dap = e.ap
                    if dap.dtype != F32:
                        cvt = sb.tile([pp, ff], F32)
                        nc.vector.tensor_copy(cvt[:, :], dap)
                        dap = cvt[:, :]
                    nc.sync.dma_start(out[0:pp, 0:ff], dap)

    if OPT["dedup_ldw"]:
        _dedup_ldweights(nc)
    if OPT["reduce_waits"]:
        _reduce_waits(nc)
    if OPT["strip"]:
        _strip_overhead(nc, entry_barrier=OPT["entry_barrier"],
                        merge_blocks=OPT["merge_blocks"])
    if split:
        _split_excess_waits(nc)
    return nc


@functools.lru_cache(maxsize=1)
def _host_constants():
    lo, hi = resize_weight_diagonals()
    u0 = (lo * SCALE).astype(np.float32)
    u1 = (hi * SCALE).astype(np.float32)
    uv = np.stack([
        np.tile(u0[None, :], (128, 1)),
        np.tile(u1[None, :], (128, 1)),
        np.tile(lo[None, :], (128, 1)),
        np.tile(hi[None, :], (128, 1)),
    ]).astype(np.float32)
    # [q, p, i] -> [p, q, i]: contiguous per-partition layout for the DMA
    return np.ascontiguousarray(uv.transpose(1, 0, 2))


def make_in_maps(x1, x2):
    x1 = np.asarray(x1)
    x2 = np.asarray(x2)
    uv = _host_constants()
    x1bf = x1.astype(BF16)
    in_maps = []
    for b in range(B):
        xb = x1bf[b]  # [64, 128, 128]
        x1arr = np.concatenate(
            [xb[:, :64].reshape(C, XHALF), xb[:, 64:].reshape(C, XHALF)], axis=0
        )  # [128, 8192]
        x2p = np.concatenate(
            [x2[b].reshape(C, PIX2), np.ones((C, PIX2P - PIX2), np.float32)],
            axis=1,
        ).astype(BF16)
        in_maps.append({
            "x1b": np.ascontiguousarray(x1arr),
            "x2c": np.ascontiguousarray(x2p),
            "uv": uv,
        })
    return in_maps


def kernel(x1, x2):
    from concourse.bass_utils import run_bass_kernel_spmd

    nc = build_program()
    in_maps = make_in_maps(x1, x2)
    res = run_bass_kernel_spmd(nc, in_maps, core_ids=list(range(NCORES)))
    outs = [np.asarray(res.results[b]["out"]) for b in range(B)]  # [I, J]
    return np.stack(outs)[:, None].astype(np.float32)
